# revision 7
# baseline (speedup 1.0000x reference)
"""Trainium2 Bass kernel: 3-level threshold activation (elementwise).

  x <  0.33          -> f32(0.333333333)  (= f32 1/3)
  0.33 <= x < 0.66   -> f32(0.6666666666) (= f32 2/3)
  x >= 0.66          -> 1.0

Exact computation in 3 elementwise passes (all output levels land exactly
after f32 rounding, so the result is bit-identical to the jnp reference):
  DVE: g = (x is_ge t1) + 1           in {1, 2}
  ACT: m = Copy(A * g)                in {A, 2A}
  DVE: out = max((x is_ge t2), m)     in {A, 2A, 1.0}

Sharding: 8192 rows split evenly across 8 NeuronCores (pure data parallel).
Memory-bound: 67.1 MB HBM traffic per core at ~358 GB/s/core.
"""

import numpy as np

import concourse.bacc as bacc
import concourse.tile as tile
from concourse import mybir
from concourse.bass_utils import run_bass_kernel_spmd

N_CORES = 8
ROWS, COLS = 8192, 8192
SHARD_ROWS = ROWS // N_CORES  # 1024
P = 128  # SBUF partitions

T1 = 0.33
T2 = 0.66
LEVEL_LO = float(np.float32(0.333333333))

_BUILT = {}


def build_nc(shard_rows: int = SHARD_ROWS, cols: int = COLS, free: int = 4096,
             bufs: int = 4, store_engine: str = "scalar"):
    nc = bacc.Bacc(
        "TRN2",
        target_bir_lowering=False,
        debug=False,
        num_devices=N_CORES,
    )
    x = nc.dram_tensor("inputs", [shard_rows, cols], mybir.dt.float32,
                       kind="ExternalInput").ap()
    o = nc.dram_tensor("out", [shard_rows, cols], mybir.dt.float32,
                       kind="ExternalOutput").ap()
    store_eng = {"scalar": nc.scalar, "sync": nc.sync}[store_engine]

    with tile.TileContext(nc) as tc:
        with tc.tile_pool(name="xp", bufs=bufs) as xp, \
             tc.tile_pool(name="gp", bufs=bufs) as gp:
            for r in range(shard_rows // P):
                for c in range(cols // free):
                    cs = slice(c * free, (c + 1) * free)
                    rs = slice(r * P, (r + 1) * P)
                    xt = xp.tile([P, free], mybir.dt.float32)
                    nc.sync.dma_start(out=xt[:], in_=x[rs, cs])
                    gt = gp.tile([P, free], mybir.dt.float32)
                    nc.vector.tensor_scalar(
                        gt[:], xt[:], T1, 1.0,
                        mybir.AluOpType.is_ge, mybir.AluOpType.add)
                    # in-place on ACT (g -> g) and DVE (x, g -> x): halves
                    # SBUF tile count so DMA queues can run deeper.
                    nc.scalar.activation(
                        gt[:], gt[:], mybir.ActivationFunctionType.Copy,
                        bias=0.0, scale=LEVEL_LO)
                    nc.vector.scalar_tensor_tensor(
                        xt[:], xt[:], T2, gt[:],
                        mybir.AluOpType.is_ge, mybir.AluOpType.max)
                    store_eng.dma_start(out=o[rs, cs], in_=xt[:])
    nc.compile()
    return nc


def _get_nc():
    if "nc" not in _BUILT:
        _BUILT["nc"] = build_nc()
    return _BUILT["nc"]


def kernel(inputs: np.ndarray, _trace: bool = False, _nc=None):
    assert inputs.shape == (ROWS, COLS) and inputs.dtype == np.float32
    nc = _nc if _nc is not None else _get_nc()
    in_maps = [
        {"inputs": np.ascontiguousarray(
            inputs[i * SHARD_ROWS:(i + 1) * SHARD_ROWS])}
        for i in range(N_CORES)
    ]
    res = run_bass_kernel_spmd(nc, in_maps, list(range(N_CORES)), trace=_trace)
    out = np.concatenate([res.results[i]["out"] for i in range(N_CORES)], axis=0)
    if _trace:
        return out, res
    return out


# revision 11
# speedup vs baseline: 1.1563x; 1.1563x over previous
"""Trainium2 Bass kernel: 3-level threshold activation (elementwise).

  x <  0.33          -> f32(0.333333333)  (= f32 1/3)
  0.33 <= x < 0.66   -> f32(0.6666666666) (= f32 2/3)
  x >= 0.66          -> 1.0

Exact 3-op decomposition (bit-identical to the jnp reference; every output
level is produced exactly, max/min introduce no rounding):

  u = (x is_ge 0.66) max (2/3)   in {2/3, 1.0}   [DVE tensor_scalar, 2 ops]
  v = (x is_ge 0.33) max (1/3)   in {1/3, 1.0}   [DVE tensor_scalar, 2 ops]
  out = min(u, v)                in {1/3, 2/3, 1.0}  [DVE tensor_tensor]

u and v depend only on x, so the per-tile dependency chain is short (2 deep)
and the Tile scheduler keeps the DMA queues saturated.

Sharding: 8192 rows split evenly across 8 NeuronCores (pure data parallel,
no communication). Memory-bound: 67.1 MB HBM traffic per core at the
per-core HBM share. Loads go out on the Sync HWDGE ring, stores on the
Scalar HWDGE ring; [128, 512] tiles with 16-deep pools keep ~16 DMAs in
flight per direction, sustaining ~390+ GB/s (measured 182-186 us).
"""

import numpy as np

import concourse.bacc as bacc
import concourse.tile as tile
from concourse import mybir
from concourse.bass_utils import run_bass_kernel_spmd

N_CORES = 8
ROWS, COLS = 8192, 8192
SHARD_ROWS = ROWS // N_CORES  # 1024
P = 128  # SBUF partitions

T1 = 0.33
T2 = 0.66
LEVEL_LO = float(np.float32(0.333333333))
LEVEL_MID = float(np.float32(0.6666666666))

_BUILT = {}


def build_nc(shard_rows: int = SHARD_ROWS, cols: int = COLS, free: int = 512,
             bufs: int = 16):
    nc = bacc.Bacc(
        "TRN2",
        target_bir_lowering=False,
        debug=False,
        num_devices=N_CORES,
    )
    x = nc.dram_tensor("inputs", [shard_rows, cols], mybir.dt.float32,
                       kind="ExternalInput").ap()
    o = nc.dram_tensor("out", [shard_rows, cols], mybir.dt.float32,
                       kind="ExternalOutput").ap()

    with tile.TileContext(nc) as tc:
        with tc.tile_pool(name="xp", bufs=bufs) as xp, \
             tc.tile_pool(name="up", bufs=bufs) as up, \
             tc.tile_pool(name="vp", bufs=bufs) as vp, \
             tc.tile_pool(name="op", bufs=bufs) as op:
            for r in range(shard_rows // P):
                for c in range(cols // free):
                    cs = slice(c * free, (c + 1) * free)
                    rs = slice(r * P, (r + 1) * P)
                    xt = xp.tile([P, free], mybir.dt.float32)
                    nc.sync.dma_start(out=xt[:], in_=x[rs, cs])
                    ut = up.tile([P, free], mybir.dt.float32)
                    nc.vector.tensor_scalar(
                        ut[:], xt[:], T2, LEVEL_MID,
                        mybir.AluOpType.is_ge, mybir.AluOpType.max)
                    vt = vp.tile([P, free], mybir.dt.float32)
                    nc.vector.tensor_scalar(
                        vt[:], xt[:], T1, LEVEL_LO,
                        mybir.AluOpType.is_ge, mybir.AluOpType.max)
                    ot = op.tile([P, free], mybir.dt.float32)
                    nc.vector.tensor_tensor(
                        ot[:], ut[:], vt[:], mybir.AluOpType.min)
                    nc.scalar.dma_start(out=o[rs, cs], in_=ot[:])
    nc.compile()
    return nc


def _get_nc():
    if "nc" not in _BUILT:
        _BUILT["nc"] = build_nc()
    return _BUILT["nc"]


def kernel(inputs: np.ndarray, _trace: bool = False, _nc=None):
    assert inputs.shape == (ROWS, COLS) and inputs.dtype == np.float32
    nc = _nc if _nc is not None else _get_nc()
    in_maps = [
        {"inputs": np.ascontiguousarray(
            inputs[i * SHARD_ROWS:(i + 1) * SHARD_ROWS])}
        for i in range(N_CORES)
    ]
    res = run_bass_kernel_spmd(nc, in_maps, list(range(N_CORES)), trace=_trace)
    out = np.concatenate([res.results[i]["out"] for i in range(N_CORES)], axis=0)
    if _trace:
        return out, res
    return out
